# revision 14
# baseline (speedup 1.0000x reference)
"""Braid causal self-attention Trainium2 kernel (8-core SPMD).

Sharding: data-parallel over batch (2) x tensor-parallel over head groups (4).
Core c handles batch b=c//4, q-heads [4g:4g+4], kv-heads [2g:2g+2], g=c%4.
Each core computes a partial projection output (Wproj input-dim shard);
partials are summed on the host (bf16 partials, fp32 host sum).

Key structure (v6):
  - q/k are only needed through the braid scores s_q/s_k: with
    g[d,t] = braid/rotary-folded weights and mh[d,t] = sqrt(cos^2+sin^2),
    s = (sum_d q*g) * rsqrt(mean_d (q*mh)^2 + eps); rotary+rmsnorm are
    never materialized. Phase 1 projects q0/q1/k for ALL time chunks
    first (the score-critical path) and defers the v projections, so the
    sigmoid stream starts much earlier.
  - attn = sigmoid(s_q[i] + s_k[j]): s_q rows are partition-broadcast by
    DMA from a DRAM bounce (no matmul, no psum), the per-key-block s_k
    column rides as the ACT bias, and the two q-heads sharing a kv head
    are stacked so one sigmoid call covers both. Causal masking is a
    128-wide paired triangular multiply on diagonal blocks only; attn@v
    uses partial-width matmuls so sub-diagonal strips are never touched.
  - All large matmuls (projections, attn@v, output projection) run in
    bf16; the braid score path stays fp32/f32r. v is transposed with the
    DMA transpose XBAR. Output projection is emitted in quarters as yt
    halves complete, overlapping the attention phase. Inputs are
    host-pre-tiled so every DMA is contiguous.
"""
import numpy as np
from contextlib import ExitStack

import ml_dtypes

import concourse.bass as bass
import concourse.mybir as mybir
import concourse.tile as tile
from concourse import bacc
from concourse.bass_utils import run_bass_kernel_spmd

F32 = mybir.dt.float32
F32R = mybir.dt.float32r
BF16 = mybir.dt.bfloat16
AF = mybir.ActivationFunctionType

T = 2048
C = 1024
D = 64
EPS = 1e-6
NCORES = 8


def build_program():
    nc = bacc.Bacc()
    dp = nc.declare_dram_parameter
    xT_d = dp("xT", [128, 4, 8, 512], BF16, isOutput=False)  # x[b].T pre-tiled
    wq_d = dp("wq", [128, 8, 256], BF16, isOutput=False)  # Wq[group].T pre-tiled
    wk_d = dp("wk", [128, 8, 128], BF16, isOutput=False)
    wv_d = dp("wv", [128, 8, 128], BF16, isOutput=False)
    wp_d = dp("wp", [128, 2, C], BF16, isOutput=False)    # Wproj[:, group].T pre-tiled (prescaled)
    gm_d = dp("gm", [128, T], F32, isOutput=False)        # braid g (2-head dup)
    mh_d = dp("mh", [128, T], F32, isOutput=False)        # sqrt(cos^2+sin^2) (2-head dup)
    sel_d = dp("sel", [128, 3, 6], F32, isOutput=False)   # head selector masks
    tri_d = dp("tri", [128, 2, 128], BF16, isOutput=False)  # tri(i>=j), 2-head dup
    out_d = dp("outp", [T, C], BF16, isOutput=True)
    out1_d = dp("outp1", [T, C], BF16, isOutput=True)

    with tile.TileContext(nc) as tc, \
         nc.allow_low_precision("bf16 matmuls fit the 2e-2 tolerance; score path stays fp32"), \
         ExitStack() as ctx:
        cons = ctx.enter_context(tc.tile_pool(name="cons", bufs=1))
        work = ctx.enter_context(tc.tile_pool(name="work", bufs=1))

        # ---- constants / weights in SBUF (DMAs spread across queues;
        # score-critical x chunks first) ----
        wq_s = cons.tile([128, 8, 256], BF16)
        wk_s = cons.tile([128, 8, 128], BF16)
        wv_s = cons.tile([128, 8, 128], BF16)
        wp_s = cons.tile([128, 2, C], BF16)
        sel_s = cons.tile([128, 3, 6], F32R)
        tri_s = cons.tile([128, 2, 128], BF16)
        eps_t = cons.tile([128, 1], F32)
        gm_s = cons.tile([128, T], F32)
        mh_s = cons.tile([128, T], F32)
        xT_s = cons.tile([128, 4, 8, 512], BF16)
        nc.sync.dma_start(out=wq_s[:], in_=wq_d.ap())
        nc.sync.dma_start(out=xT_s[:, 0], in_=xT_d.ap()[:, 0])
        nc.gpsimd.dma_start(out=xT_s[:, 1], in_=xT_d.ap()[:, 1])
        nc.sync.dma_start(out=xT_s[:, 2], in_=xT_d.ap()[:, 2])
        nc.gpsimd.dma_start(out=wk_s[:], in_=wk_d.ap())
        nc.gpsimd.dma_start(out=xT_s[:, 3], in_=xT_d.ap()[:, 3])
        nc.sync.dma_start(out=gm_s[:], in_=gm_d.ap())
        nc.gpsimd.dma_start(out=mh_s[:], in_=mh_d.ap())
        nc.gpsimd.dma_start(out=wv_s[:], in_=wv_d.ap())
        nc.sync.dma_start(out=sel_s[:], in_=sel_d.ap().bitcast(F32R))
        nc.sync.dma_start(out=tri_s[:], in_=tri_d.ap())
        nc.gpsimd.dma_start(out=wp_s[:], in_=wp_d.ap())
        nc.vector.memset(eps_t[:], EPS)

        # long-lived work tiles
        vT = work.tile([128, T], BF16)
        v_td = work.tile([128, T], BF16)  # 16 blocks of [t128, oc128]
        scomp = work.tile([6, T], F32)
        stil = work.tile([6, T], F32)     # s-tilde accumulator (pre-rsqrt)
        kcolA = work.tile([128, 2, 8], F32)   # s_k columns: [j, kh, jb] jb 0-7
        kcolB = work.tile([128, 2, 8], F32)   # s_k columns jb 8-15
        r1 = work.tile([6, T], F32)
        psq_s = work.tile([6, T], F32)
        rq = work.tile([6, T], F32)
        yt0 = work.tile([128, T], BF16)  # heads 0,1 output (d-major)
        yt1 = work.tile([128, T], BF16)

        ksc0_d = nc.dram_tensor("kscratch0", [2, 1024], F32)
        ksc1_d = nc.dram_tensor("kscratch1", [2, 1024], F32)
        sq_d = nc.dram_tensor("sqscratch", [4, T], F32)

        # ==== phase 1: projections with fused braid reductions ====
        # Score-critical tiles (q0, q1, k) for every 512-column chunk run
        # first; each chunk's braid products fold into selector matmuls
        # and the chunk's scores finish immediately (rsqrt + DRAM bounce).
        with tc.tile_pool(name="bpool", bufs=2) as bp, \
             tc.tile_pool(name="pp1", bufs=2, space="PSUM") as pp1, \
             tc.tile_pool(name="pp2", bufs=2, space="PSUM") as pp2:
            tiles = [(wq_s, 0, 0), (wq_s, 128, 1), (wk_s, 0, 2)]
            for cn in range(4):
                sl = slice(512 * cn, 512 * cn + 512)
                pss_t = pp2.tile([6, 512], F32, tag="pss")
                psq_t = pp2.tile([6, 512], F32, tag="psq")
                for w_s, oc0, t_i in tiles:
                    ps = pp1.tile([128, 512], F32, tag="pj")
                    for kt in range(8):
                        nc.tensor.matmul(
                            ps[:], w_s[:, kt, oc0:oc0 + 128],
                            xT_s[:, cn, kt, :],
                            start=(kt == 0), stop=(kt == 7))
                    a_t = bp.tile([128, 512], F32R, tag="a")
                    b_t = bp.tile([128, 512], F32, tag="b")
                    b2_t = bp.tile([128, 512], F32R, tag="b2")
                    nc.vector.tensor_mul(a_t[:], ps[:], gm_s[:, sl])
                    nc.vector.tensor_mul(b_t[:], ps[:], mh_s[:, sl])
                    nc.gpsimd.tensor_mul(b2_t[:], b_t[:], b_t[:])
                    nc.tensor.matmul(pss_t[:], sel_s[:, t_i, :], a_t[:],
                                     start=(t_i == 0), stop=(t_i == 2))
                    nc.tensor.matmul(psq_t[:], sel_s[:, t_i, :], b2_t[:],
                                     start=(t_i == 0), stop=(t_i == 2))
                # per-chunk score tail: s = stil * exp(-0.5*ln(ssq/64+eps))
                nc.vector.tensor_copy(stil[:, sl], pss_t[:])
                nc.vector.tensor_copy(psq_s[:, sl], psq_t[:])
                nc.scalar.activation(r1[:, sl], psq_s[:, sl], AF.Ln,
                                     bias=eps_t[0:6], scale=1.0 / 64.0)
                nc.scalar.activation(rq[:, sl], r1[:, sl], AF.Exp, scale=-0.5)
                nc.vector.tensor_mul(scomp[:, sl], stil[:, sl], rq[:, sl])
                kd = ksc0_d if cn < 2 else ksc1_d
                nc.sync.dma_start(out=kd.ap()[:, 512 * (cn % 2):512 * (cn % 2) + 512],
                                  in_=scomp[0:2, sl])
                nc.sync.dma_start(out=sq_d.ap()[:, sl], in_=scomp[2:6, sl])
                if cn in (1, 3):
                    kt_dst = kcolA if cn == 1 else kcolB
                    kt_src = ksc0_d if cn == 1 else ksc1_d
                    nc.gpsimd.dma_start(
                        out=kt_dst[:],
                        in_=kt_src.ap().rearrange("r (b j) -> j r b", j=128))

            # v projections (not on the score critical path) + transposes
            for cn in range(4):
                sl = slice(512 * cn, 512 * cn + 512)
                ps = pp1.tile([128, 512], F32, tag="pj")
                for kt in range(8):
                    nc.tensor.matmul(
                        ps[:], wv_s[:, kt, 0:128],
                        xT_s[:, cn, kt, :],
                        start=(kt == 0), stop=(kt == 7))
                nc.vector.tensor_copy(vT[:, sl], ps[:])
                for k in range(4):
                    jb = 4 * cn + k
                    nc.sync.dma_start(out=v_td[:, 128 * jb:128 * jb + 128],
                                      in_=vT[:, 128 * jb:128 * jb + 128],
                                      transpose=True)

        # ================= phase 3: attention + streamed projection ======
        with tc.tile_pool(name="sqpool", bufs=3) as sqp, \
             tc.tile_pool(name="atpool", bufs=8) as atp, \
             tc.tile_pool(name="ostage", bufs=4) as osp, \
             tc.tile_pool(name="pp3y", bufs=3, space="PSUM") as pp3y, \
             tc.tile_pool(name="pp4", bufs=2, space="PSUM") as pp4:
            for hs in (0, 1024):
                for kh in range(2):
                    h0 = 2 * kh
                    # s_q rows for both heads, partition-broadcast by DMA
                    sqb = sqp.tile([128, 2, 1024], F32, tag="sqb")
                    for hh in range(2):
                        qd = [nc.sync, nc.gpsimd][hh]
                        qd.dma_start(
                            out=sqb[:, hh, :],
                            in_=sq_d.ap()[h0 + hh:h0 + hh + 1, hs:hs + 1024]
                                .to_broadcast((128, 1024)))
                    jmax = (hs + 1024) // 128
                    y_ps = pp3y.tile([128, 1024], F32, tag="yps")
                    # last jb touching each 512-wide psum window
                    last_w = [min((hs + 512 * ck + 512) // 128, jmax) - 1
                              for ck in range(2)]
                    for jb in range(jmax):
                        vstart = max(hs, 128 * jb)
                        voff = vstart - hs   # first live col within the window
                        at_t = atp.tile([128, 2, 1024], BF16, tag="att")
                        # attn = sigmoid(s_q[i] + s_k[j]) for both heads in
                        # one call; s_k column as ACT bias.
                        nc.scalar.activation(
                            at_t[:, :, voff:1024],
                            sqb[:, :, voff:1024],
                            AF.Sigmoid,
                            bias=(kcolA if jb < 8 else kcolB)[:, kh, jb % 8:jb % 8 + 1])
                        # causal tri mask on the diagonal block only (pair)
                        if 128 * jb >= hs:
                            nc.vector.tensor_mul(
                                at_t[:, :, voff:voff + 128],
                                at_t[:, :, voff:voff + 128],
                                tri_s[:])
                        # attn @ v accumulation: partial-width matmuls start
                        # at the causal boundary; head hh lands on psum
                        # partitions [64*hh, 64*hh+64)
                        for ck in range(voff // 512, 2):
                            lo = max(voff, 512 * ck)
                            for hh in range(2):
                                nc.tensor.matmul(
                                    y_ps[64 * hh:64 * hh + 64, lo:512 * ck + 512],
                                    v_td[:, 128 * jb + 64 * kh:128 * jb + 64 * kh + 64],
                                    at_t[:, hh, lo:512 * ck + 512],
                                    start=(jb == 0),
                                    stop=(last_w[ck] == jb))
                    yt_dst = yt0 if kh == 0 else yt1
                    nc.vector.tensor_copy(yt_dst[:, hs:hs + 1024], y_ps[:])

                    # stream out the finished quarter of the output
                    # projection (yt[kh][:, hs:hs+1024] is now complete)
                    od = out_d if kh == 0 else out1_d
                    yt_src = yt_dst
                    tail = hs == 1024 and kh == 1
                    for ti in range(8):
                        tt = hs // 128 + ti
                        o_t = osp.tile([128, C], BF16, tag="ost")
                        for cn in range(2):
                            ps_o = pp4.tile([128, 512], F32, tag="opj")
                            nc.tensor.matmul(ps_o[:],
                                             yt_src[:, 128 * tt:128 * tt + 128],
                                             wp_s[:, kh, 512 * cn:512 * cn + 512],
                                             start=True, stop=True)
                            if tail and cn == 1:
                                nc.scalar.copy(o_t[:, 512 * cn:512 * cn + 512], ps_o[:])
                            else:
                                nc.vector.tensor_copy(o_t[:, 512 * cn:512 * cn + 512], ps_o[:])
                        qd = [nc.sync, nc.gpsimd][ti % 2]
                        qd.dma_start(
                            out=od.ap()[128 * tt:128 * tt + 128, :],
                            in_=o_t[:])

    nc.compile()
    return nc


_PROGRAM = None


def _get_program():
    global _PROGRAM
    if _PROGRAM is None:
        _PROGRAM = build_program()
    return _PROGRAM


def _host_inputs(x, cos, sin, Wq, Wk, Wv, Wproj, w_braid):
    bf = ml_dtypes.bfloat16
    cos2 = cos[:, 0, :].astype(np.float32)   # [T, 32]
    sin2 = sin[:, 0, :].astype(np.float32)
    wb = w_braid.astype(np.float32)
    g64 = np.empty((64, T), np.float32)
    g64[:32] = wb[:32, None] * cos2.T - wb[32:, None] * sin2.T
    g64[32:] = wb[32:, None] * cos2.T + wb[:32, None] * sin2.T
    gm = np.concatenate([g64, g64], axis=0)
    mh1 = np.sqrt(cos2.T ** 2 + sin2.T ** 2).astype(np.float32)  # [32, T]
    mh64 = np.concatenate([mh1, mh1], axis=0)
    mh = np.concatenate([mh64, mh64], axis=0)

    sel = np.zeros((128, 3, 6), np.float32)
    sel[0:64, 0, 2] = 1.0
    sel[64:128, 0, 3] = 1.0
    sel[0:64, 1, 4] = 1.0
    sel[64:128, 1, 5] = 1.0
    sel[0:64, 2, 0] = 1.0
    sel[64:128, 2, 1] = 1.0

    tri = (np.arange(128)[None, :] >= np.arange(128)[:, None]).astype(bf)
    pscale = np.float32(1.0 / (T ** 0.5 + 1e-6))

    in_maps = []
    for c in range(NCORES):
        b, g = c // 4, c % 4
        in_maps.append({
            "xT": np.ascontiguousarray(
                x[b].T.reshape(8, 128, 4, 512).transpose(1, 2, 0, 3)).astype(bf),
            "wq": np.ascontiguousarray(
                Wq[256 * g:256 * (g + 1)].T.reshape(8, 128, 256).transpose(1, 0, 2)).astype(bf),
            "wk": np.ascontiguousarray(
                Wk[128 * g:128 * (g + 1)].T.reshape(8, 128, 128).transpose(1, 0, 2)).astype(bf),
            "wv": np.ascontiguousarray(
                Wv[128 * g:128 * (g + 1)].T.reshape(8, 128, 128).transpose(1, 0, 2)).astype(bf),
            "wp": np.ascontiguousarray(
                (Wproj[:, 256 * g:256 * (g + 1)] * pscale).T
                .reshape(2, 128, 1024).transpose(1, 0, 2)).astype(bf),
            "gm": gm, "mh": mh, "sel": sel,
            "tri": np.ascontiguousarray(np.stack([tri, tri], axis=1)),
        })
    return in_maps


def kernel(x, cos, sin, Wq, Wk, Wv, Wproj, w_braid):
    x = np.asarray(x, np.float32)
    nc = _get_program()
    in_maps = _host_inputs(np.asarray(x, np.float32), np.asarray(cos), np.asarray(sin),
                           np.asarray(Wq, np.float32), np.asarray(Wk, np.float32),
                           np.asarray(Wv, np.float32), np.asarray(Wproj, np.float32),
                           np.asarray(w_braid, np.float32))
    res = run_bass_kernel_spmd(nc, in_maps, list(range(NCORES)))
    out = np.zeros((2, T, C), np.float32)
    for c in range(NCORES):
        out[c // 4] += res.results[c]["outp"].astype(np.float32)
        out[c // 4] += res.results[c]["outp1"].astype(np.float32)
    return out
